# revision 13
# baseline (speedup 1.0000x reference)
"""Trainium2 Bass kernel for nn_LowRankValueCached.

Computation (per reference):
  new_latent = new_x @ B^T                      (4,1,512)
  all_latent = concat([cached_latent, new_latent], 1)   (4,4097,512)
  per-row symmetric int8 quant over rank=512
  out = (q @ A_int8^T) * scale * A_scale^T      (4,4097,4096) f32

Sharding: the 16384 cached rows (batch*seq flattened) are split into 8
contiguous chunks of 2048 rows, one per core.  A_int8^T (as exact bf16
ints) is replicated.  Every core also computes the tiny 4-row "new
latent" path redundantly (B^T replicated); the host takes core 0's copy.
all_latent's cached part is a passthrough of the input, assembled on host.

Quantization on device: absmax reduce -> reciprocal -> x*(127/amax) ->
round-half-even via the +/-1.5*2^23 magic-number trick -> bf16 (exact for
ints <=127).  Matmul in bf16 is exact (products sum < 2^24, fp32 accum).
Dequant: ScalarE copy-with-per-partition-scale from PSUM, then one
VectorE tensor_mul with the partition-broadcast A_scale row.
"""

import sys
from contextlib import ExitStack

import numpy as np

sys.path.insert(0, "/opt/trn_rl_repo")

import ml_dtypes  # noqa: E402

import bass_rust as _bass_rust  # noqa: E402
import concourse.bass as bass  # noqa: E402
import concourse.tile as tile  # noqa: E402
from concourse import mybir  # noqa: E402
from concourse.bass_utils import run_bass_kernel_spmd  # noqa: E402
from concourse.vector_clock import ScopedClock  # noqa: E402

# ---------------------------------------------------------------------------
# The walrus build in this container rejects instructions carrying more than
# one semaphore wait ("Too many sync wait commands" on the kernel-tail Drain).
# Split the tail drain's wait list across a chain of serial drains on the
# sync engine, which is semantically identical.
# ---------------------------------------------------------------------------
_MAX_WAITS = 1


def _split_drain_and_barrier(self, tick_clock, wait_clock):
    drain_inst = self.nc.sync.drain()
    wait_clock.add_sem_waits(
        drain_inst.ins, ScopedClock({None: tick_clock.global_clock})
    )
    si = drain_inst.ins.sync_info
    if si is not None and si.on_wait and len(si.on_wait) > _MAX_WAITS:
        waits = list(si.on_wait)
        si.on_wait = waits[:_MAX_WAITS]
        assert self.sems is not None
        seed = next(iter(self.sems.allocated().values()))
        for i in range(_MAX_WAITS, len(waits), _MAX_WAITS):
            nop = self.nc.sync.drain()
            _bass_rust.wait_op(nop.ins, seed, 0, "sem-ge", False)
            nop.ins.sync_info.on_wait = waits[i:i + _MAX_WAITS]

    self.nc.all_engine_barrier()
    assert self.sems is not None
    popped = self.nc._tile_sem_poison_stack.pop()
    assert popped is self._sem_poison
    self.nc.clear_and_free_semaphores(list(self.sems.allocated().values()))
    self.nc.all_engine_barrier()


tile.TileContext._drain_and_barrier = _split_drain_and_barrier


def _split_multi_waits(nc, max_waits=_MAX_WAITS):
    """Walrus here allows only one sem wait per instruction.  Move excess
    waits onto NoOp carriers inserted just before the instruction on the
    same engine (engines execute their program serially, so this is
    semantically identical)."""
    n_carriers = 0
    for fn in nc.m.functions:
        for blk in fn.blocks:
            out = []
            changed = False
            for inst in blk.instructions:
                si = inst.sync_info
                if si is not None and si.on_wait and len(si.on_wait) > max_waits:
                    waits = list(si.on_wait)
                    chunks = [waits[i:i + max_waits]
                              for i in range(0, len(waits), max_waits)]
                    si.on_wait = chunks[-1]
                    for ch in chunks[:-1]:
                        nop = mybir.InstNoOp(name=f"I-{nc.next_id()}",
                                             ins=[], outs=[])
                        nop.engine = inst.engine
                        nop.sync_info = _bass_rust.SyncInfo(
                            on_wait=ch, on_update=[])
                        nc.register_instruction(nop)
                        out.append(nop)
                        n_carriers += 1
                    changed = True
                out.append(inst)
            if changed:
                blk.instructions = out
    return n_carriers

N_CORES = 8
B, S, H, R, O = 4, 4096, 4096, 512, 4096
ROWS = B * S              # 16384 cached rows
RPC = ROWS // N_CORES     # 2048 rows per core
NT = RPC // 128           # 16 tiles of 128 rows
KC = R // 128             # 4 contraction chunks
OC = O // 512             # 8 output chunks
HC = H // 128             # 32 hidden chunks (B matmul)
EPS = 1e-8
QMAX = 127.0
MAGIC = 1.5 * 2**23       # round-half-even magic constant

f32 = mybir.dt.float32
bf16 = mybir.dt.bfloat16

_NC_CACHE = {}


def _build_nc():
    nc = bass.Bass()
    cached = nc.declare_dram_parameter("cached_rows", [RPC, R], f32, isOutput=False)
    at_d = nc.declare_dram_parameter("at_sw", [128, KC * O], bf16, isOutput=False)
    asc_d = nc.declare_dram_parameter("a_scale_bc", [128, O], f32, isOutput=False)
    ident_d = nc.declare_dram_parameter("ident_bf16", [128, 128], bf16, isOutput=False)
    bt_d = nc.declare_dram_parameter("bt_sw", [128, HC * R], f32, isOutput=False)
    nxt_d = nc.declare_dram_parameter("nxt_sw", [128, HC * B], f32, isOutput=False)
    out_rows_d = nc.declare_dram_parameter("out_rows", [RPC, O], f32, isOutput=True)
    out_new_d = nc.declare_dram_parameter("out_new", [B, O], f32, isOutput=True)
    nl_d = nc.declare_dram_parameter("new_latent", [B, R], f32, isOutput=True)

    with tile.TileContext(nc) as tc, ExitStack() as ctx:
        const = ctx.enter_context(tc.tile_pool(name="const", bufs=1))
        ident = const.tile([128, 128], bf16)
        nc.sync.dma_start(ident[:], ident_d[:])

        at_sb = const.tile([128, KC * O], bf16)
        for k in range(KC):
            nc.sync.dma_start(at_sb[:, k * O:(k + 1) * O],
                              at_d[:, k * O:(k + 1) * O])
        asc_bc = const.tile([128, O], f32)
        nc.sync.dma_start(asc_bc[:], asc_d[:])
        nxt_sb = const.tile([128, HC * B], f32)
        nc.sync.dma_start(nxt_sb[:], nxt_d[:])

        work = ctx.enter_context(tc.tile_pool(name="work", bufs=3))
        stat = ctx.enter_context(tc.tile_pool(name="stat", bufs=4))
        qpool = ctx.enter_context(tc.tile_pool(name="q", bufs=3))
        outp = ctx.enter_context(tc.tile_pool(name="outp", bufs=3))
        psum_t = ctx.enter_context(tc.tile_pool(name="psum_t", bufs=2, space="PSUM"))
        psum_o = ctx.enter_context(tc.tile_pool(name="psum_o", bufs=5, space="PSUM"))
        psum_nl = ctx.enter_context(tc.tile_pool(name="psum_nl", bufs=1, space="PSUM"))

        def quant_rows(L, P, tagsuf):
            """Quantize rows of L [P, R] -> (q bf16 [P,R], sdeq f32 [P,1])."""
            absmax = stat.tile([P, 1], f32, tag="absmax" + tagsuf)
            nc.vector.tensor_reduce(
                absmax[:], L[:], axis=mybir.AxisListType.X,
                op=mybir.AluOpType.max, apply_absolute_value=True,
            )
            amax = stat.tile([P, 1], f32, tag="amax" + tagsuf)
            nc.vector.tensor_scalar_max(amax[:], absmax[:], EPS)
            inv = stat.tile([P, 1], f32, tag="inv" + tagsuf)
            nc.vector.reciprocal(inv[:], amax[:])
            inv127 = stat.tile([P, 1], f32, tag="inv127" + tagsuf)
            nc.vector.tensor_scalar_mul(inv127[:], inv[:], QMAX)
            sdeq = stat.tile([P, 1], f32, tag="sdeq" + tagsuf)
            nc.vector.tensor_scalar_mul(sdeq[:], amax[:], 1.0 / QMAX)
            t = work.tile([P, R], f32, tag="t_round" + tagsuf)
            nc.scalar.activation(
                t[:], L[:], mybir.ActivationFunctionType.Copy,
                bias=MAGIC, scale=inv127[:],
            )
            q = qpool.tile([P, R], bf16, tag="q" + tagsuf)
            nc.vector.tensor_scalar(
                q[:], t[:], MAGIC, None, mybir.AluOpType.subtract
            )
            return q, sdeq

        # ---- new-latent path: nl = new_x @ B^T, fp32 on PE ----
        btp = ctx.enter_context(tc.tile_pool(name="btp", bufs=3))
        p_nl = psum_nl.tile([B, R], f32, tag="pnl")
        for h in range(HC):
            bt_t = btp.tile([128, R], f32, tag="bt")
            nc.gpsimd.dma_start(bt_t[:], bt_d[:, h * R:(h + 1) * R])
            nc.tensor.matmul(
                p_nl[:],
                lhsT=nxt_sb[:, h * B:(h + 1) * B],
                rhs=bt_t[:],
                start=(h == 0), stop=(h == HC - 1),
            )
        nl_sb = const.tile([B, R], f32)
        nc.scalar.copy(nl_sb[:], p_nl[:])
        nc.sync.dma_start(nl_d[:], nl_sb[:])

        # ---- new-row epilogue ----
        qn, sdeq_n = quant_rows(nl_sb, B, "n")
        ptn = psum_t.tile([128, 16], bf16, tag="pt")
        qtn = qpool.tile([128, 16], bf16, tag="qtn")
        for k in range(KC):
            nc.tensor.transpose(
                ptn[:, k * B:(k + 1) * B], qn[:, k * 128:(k + 1) * 128],
                ident[:B, :B],
            )
            nc.scalar.copy(qtn[:, k * B:(k + 1) * B], ptn[:, k * B:(k + 1) * B])
        outnp = ctx.enter_context(tc.tile_pool(name="outnp", bufs=1))
        outn = outnp.tile([B, O], f32, tag="outn")
        for o in range(OC):
            pn = psum_o.tile([B, 512], f32, tag="po")
            for k in range(KC):
                nc.tensor.matmul(
                    pn[:],
                    lhsT=qtn[:, k * B:(k + 1) * B],
                    rhs=at_sb[:, k * O + o * 512: k * O + (o + 1) * 512],
                    start=(k == 0), stop=(k == KC - 1),
                )
            nc.scalar.activation(
                outn[:, o * 512:(o + 1) * 512], pn[:],
                mybir.ActivationFunctionType.Copy, scale=sdeq_n[:],
            )
        nc.vector.tensor_mul(outn[:], outn[:], asc_bc[:B, :])
        nc.sync.dma_start(out_new_d[:], outn[:])


        # ---- main tiles ----
        for t in range(NT):
            L = work.tile([128, R], f32, tag="L")
            nc.sync.dma_start(L[:], cached[t * 128:(t + 1) * 128, :])
            q, sdeq = quant_rows(L, 128, "")
            pt = psum_t.tile([128, 512], bf16, tag="pt")
            qt = qpool.tile([128, 512], bf16, tag="qt")
            for k in range(KC):
                sl = slice(k * 128, (k + 1) * 128)
                nc.tensor.transpose(pt[:, sl], q[:, sl], ident[:])
                if k % 2 == 0:
                    nc.scalar.copy(qt[:, sl], pt[:, sl])
                else:
                    nc.vector.tensor_copy(qt[:, sl], pt[:, sl])
            out_sb = outp.tile([128, O], f32, tag="out")
            for o in range(OC):
                po = psum_o.tile([128, 512], f32, tag="po")
                for k in range(KC):
                    nc.tensor.matmul(
                        po[:],
                        lhsT=qt[:, k * 128:(k + 1) * 128],
                        rhs=at_sb[:, k * O + o * 512: k * O + (o + 1) * 512],
                        start=(k == 0), stop=(k == KC - 1),
                    )
                nc.scalar.activation(
                    out_sb[:, o * 512:(o + 1) * 512], po[:],
                    mybir.ActivationFunctionType.Copy, scale=sdeq[:],
                )
            nc.vector.tensor_mul(out_sb[:], out_sb[:], asc_bc[:])
            nc.sync.dma_start(out_rows_d[t * 128:(t + 1) * 128, :], out_sb[:])

    _split_multi_waits(nc)
    return nc


def _get_nc():
    if "nc" not in _NC_CACHE:
        _NC_CACHE["nc"] = _build_nc()
    return _NC_CACHE["nc"]


def _prep_inputs(new_x, cached_latent, B_weight, A_int8, A_scale):
    at_sw = np.ascontiguousarray(
        A_int8.T.astype(ml_dtypes.bfloat16)
        .reshape(KC, 128, O).transpose(1, 0, 2).reshape(128, KC * O)
    )
    bt_sw = np.ascontiguousarray(
        B_weight.T.reshape(HC, 128, R).transpose(1, 0, 2).reshape(128, HC * R)
    ).astype(np.float32)
    nxt_sw = np.ascontiguousarray(
        new_x[:, 0, :].T.reshape(HC, 128, B).transpose(1, 0, 2).reshape(128, HC * B)
    ).astype(np.float32)
    asc_bc = np.ascontiguousarray(
        np.broadcast_to(A_scale.reshape(1, O), (128, O))).astype(np.float32)
    ident = np.eye(128, dtype=ml_dtypes.bfloat16)
    flat_cached = cached_latent.reshape(ROWS, R)
    in_maps = []
    for c in range(N_CORES):
        in_maps.append({
            "cached_rows": np.ascontiguousarray(flat_cached[c * RPC:(c + 1) * RPC]),
            "at_sw": at_sw,
            "a_scale_bc": asc_bc,
            "ident_bf16": ident,
            "bt_sw": bt_sw,
            "nxt_sw": nxt_sw,
        })
    return in_maps


def run_device(new_x, cached_latent, B_weight, A_int8, A_scale, trace=False):
    nc = _get_nc()
    in_maps = _prep_inputs(new_x, cached_latent, B_weight, A_int8, A_scale)
    res = run_bass_kernel_spmd(nc, in_maps, list(range(N_CORES)), trace=trace)
    out = np.empty((B, S + 1, O), np.float32)
    for c in range(N_CORES):
        b0 = (c * RPC) // S
        s0 = (c * RPC) % S
        out[b0, s0:s0 + RPC, :] = res.results[c]["out_rows"]
    out[:, S, :] = res.results[0]["out_new"]
    all_latent = np.concatenate(
        [cached_latent.astype(np.float32),
         res.results[0]["new_latent"].reshape(B, 1, R)], axis=1
    )
    return (out, all_latent), res


def kernel(new_x, cached_latent, B_weight, A_int8, A_scale):
    (out, all_latent), _ = run_device(
        np.asarray(new_x), np.asarray(cached_latent), np.asarray(B_weight),
        np.asarray(A_int8), np.asarray(A_scale),
    )
    return out, all_latent


# revision 14
# speedup vs baseline: 1.0967x; 1.0967x over previous
"""Trainium2 Bass kernel for nn_LowRankValueCached.

Computation (per reference):
  new_latent = new_x @ B^T                      (4,1,512)
  all_latent = concat([cached_latent, new_latent], 1)   (4,4097,512)
  per-row symmetric int8 quant over rank=512
  out = (q @ A_int8^T) * scale * A_scale^T      (4,4097,4096) f32

Sharding: the 16384 cached rows (batch*seq flattened) are split into 8
contiguous chunks of 2048 rows, one per core.  A_int8^T (as exact bf16
ints) is replicated.  Every core also computes the tiny 4-row "new
latent" path redundantly (B^T replicated); the host takes core 0's copy.
all_latent's cached part is a passthrough of the input, assembled on host.

Quantization on device: absmax reduce -> reciprocal -> x*(127/amax) ->
round-half-even via the +/-1.5*2^23 magic-number trick -> bf16 (exact for
ints <=127).  Matmul in bf16 is exact (products sum < 2^24, fp32 accum).
Dequant: ScalarE copy-with-per-partition-scale from PSUM, then one
VectorE tensor_mul with the partition-broadcast A_scale row.
"""

import sys
from contextlib import ExitStack

import numpy as np

sys.path.insert(0, "/opt/trn_rl_repo")

import ml_dtypes  # noqa: E402

import bass_rust as _bass_rust  # noqa: E402
import concourse.bass as bass  # noqa: E402
import concourse.tile as tile  # noqa: E402
from concourse import mybir  # noqa: E402
from concourse.bass_utils import run_bass_kernel_spmd  # noqa: E402
from concourse.vector_clock import ScopedClock  # noqa: E402

# ---------------------------------------------------------------------------
# The walrus build in this container rejects instructions carrying more than
# one semaphore wait ("Too many sync wait commands" on the kernel-tail Drain).
# Split the tail drain's wait list across a chain of serial drains on the
# sync engine, which is semantically identical.
# ---------------------------------------------------------------------------
_MAX_WAITS = 1


def _split_drain_and_barrier(self, tick_clock, wait_clock):
    drain_inst = self.nc.sync.drain()
    wait_clock.add_sem_waits(
        drain_inst.ins, ScopedClock({None: tick_clock.global_clock})
    )
    si = drain_inst.ins.sync_info
    if si is not None and si.on_wait and len(si.on_wait) > _MAX_WAITS:
        waits = list(si.on_wait)
        si.on_wait = waits[:_MAX_WAITS]
        assert self.sems is not None
        seed = next(iter(self.sems.allocated().values()))
        for i in range(_MAX_WAITS, len(waits), _MAX_WAITS):
            nop = self.nc.sync.drain()
            _bass_rust.wait_op(nop.ins, seed, 0, "sem-ge", False)
            nop.ins.sync_info.on_wait = waits[i:i + _MAX_WAITS]

    self.nc.all_engine_barrier()
    assert self.sems is not None
    popped = self.nc._tile_sem_poison_stack.pop()
    assert popped is self._sem_poison
    self.nc.clear_and_free_semaphores(list(self.sems.allocated().values()))
    self.nc.all_engine_barrier()


tile.TileContext._drain_and_barrier = _split_drain_and_barrier


def _split_multi_waits(nc, max_waits=_MAX_WAITS):
    """Walrus here allows only one sem wait per instruction.  Move excess
    waits onto NoOp carriers inserted just before the instruction on the
    same engine (engines execute their program serially, so this is
    semantically identical)."""
    n_carriers = 0
    for fn in nc.m.functions:
        for blk in fn.blocks:
            out = []
            changed = False
            for inst in blk.instructions:
                si = inst.sync_info
                if si is not None and si.on_wait and len(si.on_wait) > max_waits:
                    waits = list(si.on_wait)
                    chunks = [waits[i:i + max_waits]
                              for i in range(0, len(waits), max_waits)]
                    si.on_wait = chunks[-1]
                    for ch in chunks[:-1]:
                        nop = mybir.InstNoOp(name=f"I-{nc.next_id()}",
                                             ins=[], outs=[])
                        nop.engine = inst.engine
                        nop.sync_info = _bass_rust.SyncInfo(
                            on_wait=ch, on_update=[])
                        nc.register_instruction(nop)
                        out.append(nop)
                        n_carriers += 1
                    changed = True
                out.append(inst)
            if changed:
                blk.instructions = out
    return n_carriers

N_CORES = 8
B, S, H, R, O = 4, 4096, 4096, 512, 4096
ROWS = B * S              # 16384 cached rows
RPC = ROWS // N_CORES     # 2048 rows per core
NT = RPC // 128           # 16 tiles of 128 rows
KC = R // 128             # 4 contraction chunks
OC = O // 512             # 8 output chunks
HC = H // 128             # 32 hidden chunks (B matmul)
EPS = 1e-8
QMAX = 127.0
MAGIC = 1.5 * 2**23       # round-half-even magic constant

f32 = mybir.dt.float32
bf16 = mybir.dt.bfloat16

_NC_CACHE = {}


def _build_nc():
    nc = bass.Bass()
    cached = nc.declare_dram_parameter("cached_rows", [RPC, R], f32, isOutput=False)
    at_d = nc.declare_dram_parameter("at_sw", [128, KC * O], bf16, isOutput=False)
    asc_d = nc.declare_dram_parameter("a_scale_bc", [128, O], f32, isOutput=False)
    ident_d = nc.declare_dram_parameter("ident_bf16", [128, 128], bf16, isOutput=False)
    bt_d = nc.declare_dram_parameter("bt_sw", [128, HC * R], f32, isOutput=False)
    nxt_d = nc.declare_dram_parameter("nxt_sw", [128, HC * B], f32, isOutput=False)
    out_rows_d = nc.declare_dram_parameter("out_rows", [RPC, O], f32, isOutput=True)
    out_new_d = nc.declare_dram_parameter("out_new", [B, O], f32, isOutput=True)
    nl_d = nc.declare_dram_parameter("new_latent", [B, R], f32, isOutput=True)

    with tile.TileContext(nc) as tc, ExitStack() as ctx:
        const = ctx.enter_context(tc.tile_pool(name="const", bufs=1))
        ident = const.tile([128, 128], bf16)
        nc.sync.dma_start(ident[:], ident_d[:])

        at_sb = const.tile([128, KC * O], bf16)
        for k in range(KC):
            nc.sync.dma_start(at_sb[:, k * O:(k + 1) * O],
                              at_d[:, k * O:(k + 1) * O])
        asc_bc = const.tile([128, O], f32)
        nc.sync.dma_start(asc_bc[:], asc_d[:])
        nxt_sb = const.tile([128, HC * B], f32)
        nc.sync.dma_start(nxt_sb[:], nxt_d[:])

        work = ctx.enter_context(tc.tile_pool(name="work", bufs=3))
        stat = ctx.enter_context(tc.tile_pool(name="stat", bufs=4))
        qpool = ctx.enter_context(tc.tile_pool(name="q", bufs=3))
        outp = ctx.enter_context(tc.tile_pool(name="outp", bufs=3))
        psum_t = ctx.enter_context(tc.tile_pool(name="psum_t", bufs=2, space="PSUM"))
        psum_o = ctx.enter_context(tc.tile_pool(name="psum_o", bufs=5, space="PSUM"))
        psum_nl = ctx.enter_context(tc.tile_pool(name="psum_nl", bufs=1, space="PSUM"))

        def quant_rows(L, P, tagsuf):
            """Quantize rows of L [P, R] -> (q bf16 [P,R], sdeq f32 [P,1])."""
            absmax = stat.tile([P, 1], f32, tag="absmax" + tagsuf)
            nc.vector.tensor_reduce(
                absmax[:], L[:], axis=mybir.AxisListType.X,
                op=mybir.AluOpType.max, apply_absolute_value=True,
            )
            amax = stat.tile([P, 1], f32, tag="amax" + tagsuf)
            nc.vector.tensor_scalar_max(amax[:], absmax[:], EPS)
            inv = stat.tile([P, 1], f32, tag="inv" + tagsuf)
            nc.vector.reciprocal(inv[:], amax[:])
            inv127 = stat.tile([P, 1], f32, tag="inv127" + tagsuf)
            nc.vector.tensor_scalar_mul(inv127[:], inv[:], QMAX)
            sdeq = stat.tile([P, 1], f32, tag="sdeq" + tagsuf)
            nc.vector.tensor_scalar_mul(sdeq[:], amax[:], 1.0 / QMAX)
            t = work.tile([P, R], f32, tag="t_round" + tagsuf)
            nc.scalar.activation(
                t[:], L[:], mybir.ActivationFunctionType.Copy,
                bias=MAGIC, scale=inv127[:],
            )
            q = qpool.tile([P, R], bf16, tag="q" + tagsuf)
            nc.vector.tensor_scalar(
                q[:], t[:], MAGIC, None, mybir.AluOpType.subtract
            )
            return q, sdeq

        # ---- new-latent path: nl = new_x @ B^T, fp32 on PE ----
        btp = ctx.enter_context(tc.tile_pool(name="btp", bufs=3))
        p_nl = psum_nl.tile([B, R], f32, tag="pnl")
        for h in range(HC):
            bt_t = btp.tile([128, R], f32, tag="bt")
            nc.gpsimd.dma_start(bt_t[:], bt_d[:, h * R:(h + 1) * R])
            nc.tensor.matmul(
                p_nl[:],
                lhsT=nxt_sb[:, h * B:(h + 1) * B],
                rhs=bt_t[:],
                start=(h == 0), stop=(h == HC - 1),
            )
        nl_sb = const.tile([B, R], f32)
        nc.scalar.copy(nl_sb[:], p_nl[:])
        nc.sync.dma_start(nl_d[:], nl_sb[:])

        # ---- main tiles ----
        for t in range(NT):
            L = work.tile([128, R], f32, tag="L")
            nc.sync.dma_start(L[:], cached[t * 128:(t + 1) * 128, :])
            q, sdeq = quant_rows(L, 128, "")
            pt = psum_t.tile([128, 512], bf16, tag="pt")
            qt = qpool.tile([128, 512], bf16, tag="qt")
            for k in range(KC):
                sl = slice(k * 128, (k + 1) * 128)
                nc.tensor.transpose(pt[:, sl], q[:, sl], ident[:])
                if k % 2 == 0:
                    nc.scalar.copy(qt[:, sl], pt[:, sl])
                else:
                    nc.vector.tensor_copy(qt[:, sl], pt[:, sl])
            out_sb = outp.tile([128, O], f32, tag="out")
            for o in range(OC):
                po = psum_o.tile([128, 512], f32, tag="po")
                for k in range(KC):
                    nc.tensor.matmul(
                        po[:],
                        lhsT=qt[:, k * 128:(k + 1) * 128],
                        rhs=at_sb[:, k * O + o * 512: k * O + (o + 1) * 512],
                        start=(k == 0), stop=(k == KC - 1),
                    )
                nc.scalar.activation(
                    out_sb[:, o * 512:(o + 1) * 512], po[:],
                    mybir.ActivationFunctionType.Copy, scale=sdeq[:],
                )
            nc.vector.tensor_mul(out_sb[:], out_sb[:], asc_bc[:])
            nc.sync.dma_start(out_rows_d[t * 128:(t + 1) * 128, :], out_sb[:])

        # ---- new-row epilogue ----
        qn, sdeq_n = quant_rows(nl_sb, B, "n")
        ptn = psum_t.tile([128, 16], bf16, tag="pt")
        qtn = qpool.tile([128, 16], bf16, tag="qtn")
        for k in range(KC):
            nc.tensor.transpose(
                ptn[:, k * B:(k + 1) * B], qn[:, k * 128:(k + 1) * 128],
                ident[:B, :B],
            )
            nc.scalar.copy(qtn[:, k * B:(k + 1) * B], ptn[:, k * B:(k + 1) * B])
        outnp = ctx.enter_context(tc.tile_pool(name="outnp", bufs=1))
        outn = outnp.tile([B, O], f32, tag="outn")
        for o in range(OC):
            pn = psum_o.tile([B, 512], f32, tag="po")
            for k in range(KC):
                nc.tensor.matmul(
                    pn[:],
                    lhsT=qtn[:, k * B:(k + 1) * B],
                    rhs=at_sb[:, k * O + o * 512: k * O + (o + 1) * 512],
                    start=(k == 0), stop=(k == KC - 1),
                )
            nc.scalar.activation(
                outn[:, o * 512:(o + 1) * 512], pn[:],
                mybir.ActivationFunctionType.Copy, scale=sdeq_n[:],
            )
        nc.vector.tensor_mul(outn[:], outn[:], asc_bc[:B, :])
        nc.sync.dma_start(out_new_d[:], outn[:])



    _split_multi_waits(nc)
    return nc


def _get_nc():
    if "nc" not in _NC_CACHE:
        _NC_CACHE["nc"] = _build_nc()
    return _NC_CACHE["nc"]


def _prep_inputs(new_x, cached_latent, B_weight, A_int8, A_scale):
    at_sw = np.ascontiguousarray(
        A_int8.T.astype(ml_dtypes.bfloat16)
        .reshape(KC, 128, O).transpose(1, 0, 2).reshape(128, KC * O)
    )
    bt_sw = np.ascontiguousarray(
        B_weight.T.reshape(HC, 128, R).transpose(1, 0, 2).reshape(128, HC * R)
    ).astype(np.float32)
    nxt_sw = np.ascontiguousarray(
        new_x[:, 0, :].T.reshape(HC, 128, B).transpose(1, 0, 2).reshape(128, HC * B)
    ).astype(np.float32)
    asc_bc = np.ascontiguousarray(
        np.broadcast_to(A_scale.reshape(1, O), (128, O))).astype(np.float32)
    ident = np.eye(128, dtype=ml_dtypes.bfloat16)
    flat_cached = cached_latent.reshape(ROWS, R)
    in_maps = []
    for c in range(N_CORES):
        in_maps.append({
            "cached_rows": np.ascontiguousarray(flat_cached[c * RPC:(c + 1) * RPC]),
            "at_sw": at_sw,
            "a_scale_bc": asc_bc,
            "ident_bf16": ident,
            "bt_sw": bt_sw,
            "nxt_sw": nxt_sw,
        })
    return in_maps


def run_device(new_x, cached_latent, B_weight, A_int8, A_scale, trace=False):
    nc = _get_nc()
    in_maps = _prep_inputs(new_x, cached_latent, B_weight, A_int8, A_scale)
    res = run_bass_kernel_spmd(nc, in_maps, list(range(N_CORES)), trace=trace)
    out = np.empty((B, S + 1, O), np.float32)
    for c in range(N_CORES):
        b0 = (c * RPC) // S
        s0 = (c * RPC) % S
        out[b0, s0:s0 + RPC, :] = res.results[c]["out_rows"]
    out[:, S, :] = res.results[0]["out_new"]
    all_latent = np.concatenate(
        [cached_latent.astype(np.float32),
         res.results[0]["new_latent"].reshape(B, 1, R)], axis=1
    )
    return (out, all_latent), res


def kernel(new_x, cached_latent, B_weight, A_int8, A_scale):
    (out, all_latent), _ = run_device(
        np.asarray(new_x), np.asarray(cached_latent), np.asarray(B_weight),
        np.asarray(A_int8), np.asarray(A_scale),
    )
    return out, all_latent


# revision 15
# speedup vs baseline: 1.1893x; 1.0845x over previous
"""Trainium2 Bass kernel for nn_LowRankValueCached.

Computation (per reference):
  new_latent = new_x @ B^T                      (4,1,512)
  all_latent = concat([cached_latent, new_latent], 1)   (4,4097,512)
  per-row symmetric int8 quant over rank=512
  out = (q @ A_int8^T) * scale * A_scale^T      (4,4097,4096) f32

Sharding: the 16384 cached rows (batch*seq flattened) are split into 8
contiguous chunks of 2048 rows, one per core.  A_int8^T (as exact bf16
ints) is replicated.  Every core also computes the tiny 4-row "new
latent" path redundantly (B^T replicated); the host takes core 0's copy.
all_latent's cached part is a passthrough of the input, assembled on host.

Quantization on device: absmax reduce -> reciprocal -> x*(127/amax) ->
round-half-even via the +/-1.5*2^23 magic-number trick -> bf16 (exact for
ints <=127).  Matmul in bf16 is exact (products sum < 2^24, fp32 accum).
Dequant: ScalarE copy-with-per-partition-scale from PSUM, then one
VectorE tensor_mul with the partition-broadcast A_scale row.
"""

import sys
from contextlib import ExitStack

import numpy as np

sys.path.insert(0, "/opt/trn_rl_repo")

import ml_dtypes  # noqa: E402

import bass_rust as _bass_rust  # noqa: E402
import concourse.bass as bass  # noqa: E402
import concourse.tile as tile  # noqa: E402
from concourse import mybir  # noqa: E402
from concourse.bass_utils import run_bass_kernel_spmd  # noqa: E402
from concourse.vector_clock import ScopedClock  # noqa: E402

# ---------------------------------------------------------------------------
# The walrus build in this container rejects instructions carrying more than
# one semaphore wait ("Too many sync wait commands" on the kernel-tail Drain).
# Split the tail drain's wait list across a chain of serial drains on the
# sync engine, which is semantically identical.
# ---------------------------------------------------------------------------
_MAX_WAITS = 1


def _split_drain_and_barrier(self, tick_clock, wait_clock):
    drain_inst = self.nc.sync.drain()
    wait_clock.add_sem_waits(
        drain_inst.ins, ScopedClock({None: tick_clock.global_clock})
    )
    si = drain_inst.ins.sync_info
    if si is not None and si.on_wait and len(si.on_wait) > _MAX_WAITS:
        waits = list(si.on_wait)
        si.on_wait = waits[:_MAX_WAITS]
        assert self.sems is not None
        seed = next(iter(self.sems.allocated().values()))
        for i in range(_MAX_WAITS, len(waits), _MAX_WAITS):
            nop = self.nc.sync.drain()
            _bass_rust.wait_op(nop.ins, seed, 0, "sem-ge", False)
            nop.ins.sync_info.on_wait = waits[i:i + _MAX_WAITS]

    self.nc.all_engine_barrier()
    assert self.sems is not None
    popped = self.nc._tile_sem_poison_stack.pop()
    assert popped is self._sem_poison
    self.nc.clear_and_free_semaphores(list(self.sems.allocated().values()))
    self.nc.all_engine_barrier()


tile.TileContext._drain_and_barrier = _split_drain_and_barrier


def _split_multi_waits(nc, max_waits=_MAX_WAITS):
    """Walrus here allows only one sem wait per instruction.  Move excess
    waits onto NoOp carriers inserted just before the instruction on the
    same engine (engines execute their program serially, so this is
    semantically identical)."""
    n_carriers = 0
    for fn in nc.m.functions:
        for blk in fn.blocks:
            out = []
            changed = False
            for inst in blk.instructions:
                si = inst.sync_info
                if si is not None and si.on_wait and len(si.on_wait) > max_waits:
                    waits = list(si.on_wait)
                    chunks = [waits[i:i + max_waits]
                              for i in range(0, len(waits), max_waits)]
                    si.on_wait = chunks[-1]
                    for ch in chunks[:-1]:
                        nop = mybir.InstNoOp(name=f"I-{nc.next_id()}",
                                             ins=[], outs=[])
                        nop.engine = inst.engine
                        nop.sync_info = _bass_rust.SyncInfo(
                            on_wait=ch, on_update=[])
                        nc.register_instruction(nop)
                        out.append(nop)
                        n_carriers += 1
                    changed = True
                out.append(inst)
            if changed:
                blk.instructions = out
    return n_carriers

N_CORES = 8
B, S, H, R, O = 4, 4096, 4096, 512, 4096
ROWS = B * S              # 16384 cached rows
RPC = ROWS // N_CORES     # 2048 rows per core
NT = RPC // 128           # 16 tiles of 128 rows
KC = R // 128             # 4 contraction chunks
OC = O // 512             # 8 output chunks
HC = H // 128             # 32 hidden chunks (B matmul)
EPS = 1e-8
QMAX = 127.0
MAGIC = 1.5 * 2**23       # round-half-even magic constant

f32 = mybir.dt.float32
bf16 = mybir.dt.bfloat16

_NC_CACHE = {}


def _build_nc():
    nc = bass.Bass()
    cached = nc.declare_dram_parameter("cached_rows", [RPC, R], f32, isOutput=False)
    at_d = nc.declare_dram_parameter("at_sw", [128, KC * O], bf16, isOutput=False)
    asc_d = nc.declare_dram_parameter("a_scale_bc", [128, O], f32, isOutput=False)
    ident_d = nc.declare_dram_parameter("ident_bf16", [128, 128], bf16, isOutput=False)
    bt_d = nc.declare_dram_parameter("bt_sw", [128, HC * R], f32, isOutput=False)
    nxt_d = nc.declare_dram_parameter("nxt_sw", [128, HC * B], f32, isOutput=False)
    out_rows_d = nc.declare_dram_parameter("out_rows", [RPC, O], f32, isOutput=True)
    out_new_d = nc.declare_dram_parameter("out_new", [B, O], f32, isOutput=True)
    nl_d = nc.declare_dram_parameter("new_latent", [B, R], f32, isOutput=True)

    with tile.TileContext(nc) as tc, ExitStack() as ctx:
        const = ctx.enter_context(tc.tile_pool(name="const", bufs=1))
        ident = const.tile([128, 128], bf16)
        nc.sync.dma_start(ident[:], ident_d[:])

        at_sb = const.tile([128, KC * O], bf16)
        for k in range(KC):
            nc.sync.dma_start(at_sb[:, k * O:(k + 1) * O],
                              at_d[:, k * O:(k + 1) * O])
        asc_bc = const.tile([128, O], f32)
        nc.sync.dma_start(asc_bc[:], asc_d[:])
        nxt_sb = const.tile([128, HC * B], f32)
        nc.sync.dma_start(nxt_sb[:], nxt_d[:])

        work = ctx.enter_context(tc.tile_pool(name="work", bufs=3))
        stat = ctx.enter_context(tc.tile_pool(name="stat", bufs=4))
        qpool = ctx.enter_context(tc.tile_pool(name="q", bufs=3))
        outp = ctx.enter_context(tc.tile_pool(name="outp", bufs=3))
        psum_t = ctx.enter_context(tc.tile_pool(name="psum_t", bufs=2, space="PSUM"))
        psum_o = ctx.enter_context(tc.tile_pool(name="psum_o", bufs=5, space="PSUM"))
        psum_nl = ctx.enter_context(tc.tile_pool(name="psum_nl", bufs=1, space="PSUM"))

        def quant_rows(L, P, tagsuf):
            """Quantize rows of L [P, R] -> (q bf16 [P,R], sdeq f32 [P,1])."""
            absmax = stat.tile([P, 1], f32, tag="absmax" + tagsuf)
            nc.vector.tensor_reduce(
                absmax[:], L[:], axis=mybir.AxisListType.X,
                op=mybir.AluOpType.max, apply_absolute_value=True,
            )
            amax = stat.tile([P, 1], f32, tag="amax" + tagsuf)
            nc.vector.tensor_scalar_max(amax[:], absmax[:], EPS)
            inv = stat.tile([P, 1], f32, tag="inv" + tagsuf)
            nc.vector.reciprocal(inv[:], amax[:])
            inv127 = stat.tile([P, 1], f32, tag="inv127" + tagsuf)
            nc.vector.tensor_scalar_mul(inv127[:], inv[:], QMAX)
            sdeq = stat.tile([P, 1], f32, tag="sdeq" + tagsuf)
            nc.vector.tensor_scalar_mul(sdeq[:], amax[:], 1.0 / QMAX)
            t = work.tile([P, R], f32, tag="t_round" + tagsuf)
            nc.scalar.activation(
                t[:], L[:], mybir.ActivationFunctionType.Copy,
                bias=MAGIC, scale=inv127[:],
            )
            q = qpool.tile([P, R], bf16, tag="q" + tagsuf)
            nc.vector.tensor_scalar(
                q[:], t[:], MAGIC, None, mybir.AluOpType.subtract
            )
            return q, sdeq

        # ---- new-latent path: nl = new_x @ B^T, fp32 on PE ----
        btp = ctx.enter_context(tc.tile_pool(name="btp", bufs=3))
        p_nl = psum_nl.tile([B, R], f32, tag="pnl")
        for h in range(HC):
            bt_t = btp.tile([128, R], f32, tag="bt")
            nc.sync.dma_start(bt_t[:], bt_d[:, h * R:(h + 1) * R])
            nc.tensor.matmul(
                p_nl[:],
                lhsT=nxt_sb[:, h * B:(h + 1) * B],
                rhs=bt_t[:],
                start=(h == 0), stop=(h == HC - 1),
            )
        nl_sb = const.tile([B, R], f32)
        nc.scalar.copy(nl_sb[:], p_nl[:])
        nc.sync.dma_start(nl_d[:], nl_sb[:])

        # ---- main tiles ----
        for t in range(NT):
            L = work.tile([128, R], f32, tag="L")
            nc.sync.dma_start(L[:], cached[t * 128:(t + 1) * 128, :])
            q, sdeq = quant_rows(L, 128, "")
            pt = psum_t.tile([128, 512], bf16, tag="pt")
            qt = qpool.tile([128, 512], bf16, tag="qt")
            for k in range(KC):
                sl = slice(k * 128, (k + 1) * 128)
                nc.tensor.transpose(pt[:, sl], q[:, sl], ident[:])
                if k % 2 == 0:
                    nc.scalar.copy(qt[:, sl], pt[:, sl])
                else:
                    nc.vector.tensor_copy(qt[:, sl], pt[:, sl])
            out_sb = outp.tile([128, O], f32, tag="out")
            for o in range(OC):
                po = psum_o.tile([128, 512], f32, tag="po")
                for k in range(KC):
                    nc.tensor.matmul(
                        po[:],
                        lhsT=qt[:, k * 128:(k + 1) * 128],
                        rhs=at_sb[:, k * O + o * 512: k * O + (o + 1) * 512],
                        start=(k == 0), stop=(k == KC - 1),
                    )
                nc.scalar.activation(
                    out_sb[:, o * 512:(o + 1) * 512], po[:],
                    mybir.ActivationFunctionType.Copy, scale=sdeq[:],
                )
            nc.vector.tensor_mul(out_sb[:], out_sb[:], asc_bc[:])
            nc.sync.dma_start(out_rows_d[t * 128:(t + 1) * 128, :], out_sb[:])

        # ---- new-row epilogue ----
        qn, sdeq_n = quant_rows(nl_sb, B, "n")
        ptn = psum_t.tile([128, 16], bf16, tag="pt")
        qtn = qpool.tile([128, 16], bf16, tag="qtn")
        for k in range(KC):
            nc.tensor.transpose(
                ptn[:, k * B:(k + 1) * B], qn[:, k * 128:(k + 1) * 128],
                ident[:B, :B],
            )
            nc.scalar.copy(qtn[:, k * B:(k + 1) * B], ptn[:, k * B:(k + 1) * B])
        outnp = ctx.enter_context(tc.tile_pool(name="outnp", bufs=1))
        outn = outnp.tile([B, O], f32, tag="outn")
        for o in range(OC):
            pn = psum_o.tile([B, 512], f32, tag="po")
            for k in range(KC):
                nc.tensor.matmul(
                    pn[:],
                    lhsT=qtn[:, k * B:(k + 1) * B],
                    rhs=at_sb[:, k * O + o * 512: k * O + (o + 1) * 512],
                    start=(k == 0), stop=(k == KC - 1),
                )
            nc.scalar.activation(
                outn[:, o * 512:(o + 1) * 512], pn[:],
                mybir.ActivationFunctionType.Copy, scale=sdeq_n[:],
            )
        nc.vector.tensor_mul(outn[:], outn[:], asc_bc[:B, :])
        nc.sync.dma_start(out_new_d[:], outn[:])



    _split_multi_waits(nc)
    return nc


def _get_nc():
    if "nc" not in _NC_CACHE:
        _NC_CACHE["nc"] = _build_nc()
    return _NC_CACHE["nc"]


def _prep_inputs(new_x, cached_latent, B_weight, A_int8, A_scale):
    at_sw = np.ascontiguousarray(
        A_int8.T.astype(ml_dtypes.bfloat16)
        .reshape(KC, 128, O).transpose(1, 0, 2).reshape(128, KC * O)
    )
    bt_sw = np.ascontiguousarray(
        B_weight.T.reshape(HC, 128, R).transpose(1, 0, 2).reshape(128, HC * R)
    ).astype(np.float32)
    nxt_sw = np.ascontiguousarray(
        new_x[:, 0, :].T.reshape(HC, 128, B).transpose(1, 0, 2).reshape(128, HC * B)
    ).astype(np.float32)
    asc_bc = np.ascontiguousarray(
        np.broadcast_to(A_scale.reshape(1, O), (128, O))).astype(np.float32)
    ident = np.eye(128, dtype=ml_dtypes.bfloat16)
    flat_cached = cached_latent.reshape(ROWS, R)
    in_maps = []
    for c in range(N_CORES):
        in_maps.append({
            "cached_rows": np.ascontiguousarray(flat_cached[c * RPC:(c + 1) * RPC]),
            "at_sw": at_sw,
            "a_scale_bc": asc_bc,
            "ident_bf16": ident,
            "bt_sw": bt_sw,
            "nxt_sw": nxt_sw,
        })
    return in_maps


def run_device(new_x, cached_latent, B_weight, A_int8, A_scale, trace=False):
    nc = _get_nc()
    in_maps = _prep_inputs(new_x, cached_latent, B_weight, A_int8, A_scale)
    res = run_bass_kernel_spmd(nc, in_maps, list(range(N_CORES)), trace=trace)
    out = np.empty((B, S + 1, O), np.float32)
    for c in range(N_CORES):
        b0 = (c * RPC) // S
        s0 = (c * RPC) % S
        out[b0, s0:s0 + RPC, :] = res.results[c]["out_rows"]
    out[:, S, :] = res.results[0]["out_new"]
    all_latent = np.concatenate(
        [cached_latent.astype(np.float32),
         res.results[0]["new_latent"].reshape(B, 1, R)], axis=1
    )
    return (out, all_latent), res


def kernel(new_x, cached_latent, B_weight, A_int8, A_scale):
    (out, all_latent), _ = run_device(
        np.asarray(new_x), np.asarray(cached_latent), np.asarray(B_weight),
        np.asarray(A_int8), np.asarray(A_scale),
    )
    return out, all_latent


# revision 16
# speedup vs baseline: 1.3884x; 1.1674x over previous
"""Trainium2 Bass kernel for nn_LowRankValueCached.

Computation (per reference):
  new_latent = new_x @ B^T                      (4,1,512)
  all_latent = concat([cached_latent, new_latent], 1)   (4,4097,512)
  per-row symmetric int8 quant over rank=512
  out = (q @ A_int8^T) * scale * A_scale^T      (4,4097,4096) f32

Sharding: the 16384 cached rows (batch*seq flattened) are split into 8
contiguous chunks of 2048 rows, one per core.  A_int8^T (as exact bf16
ints) is replicated.  Every core also computes the tiny 4-row "new
latent" path redundantly (B^T replicated); the host takes core 0's copy.
all_latent's cached part is a passthrough of the input, assembled on host.

Quantization on device: absmax reduce -> reciprocal -> x*(127/amax) ->
round-half-even via the +/-1.5*2^23 magic-number trick -> bf16 (exact for
ints <=127).  Matmul in bf16 is exact (products sum < 2^24, fp32 accum).
Dequant: ScalarE copy-with-per-partition-scale from PSUM, then one
VectorE tensor_mul with the partition-broadcast A_scale row.
"""

import sys
from contextlib import ExitStack

import numpy as np

sys.path.insert(0, "/opt/trn_rl_repo")

import ml_dtypes  # noqa: E402

import bass_rust as _bass_rust  # noqa: E402
import concourse.bass as bass  # noqa: E402
import concourse.tile as tile  # noqa: E402
from concourse import mybir  # noqa: E402
from concourse.bass_utils import run_bass_kernel_spmd  # noqa: E402
from concourse.vector_clock import ScopedClock  # noqa: E402

# ---------------------------------------------------------------------------
# The walrus build in this container rejects instructions carrying more than
# one semaphore wait ("Too many sync wait commands" on the kernel-tail Drain).
# Split the tail drain's wait list across a chain of serial drains on the
# sync engine, which is semantically identical.
# ---------------------------------------------------------------------------
_MAX_WAITS = 1


def _split_drain_and_barrier(self, tick_clock, wait_clock):
    drain_inst = self.nc.sync.drain()
    wait_clock.add_sem_waits(
        drain_inst.ins, ScopedClock({None: tick_clock.global_clock})
    )
    si = drain_inst.ins.sync_info
    if si is not None and si.on_wait and len(si.on_wait) > _MAX_WAITS:
        waits = list(si.on_wait)
        si.on_wait = waits[:_MAX_WAITS]
        assert self.sems is not None
        seed = next(iter(self.sems.allocated().values()))
        for i in range(_MAX_WAITS, len(waits), _MAX_WAITS):
            nop = self.nc.sync.drain()
            _bass_rust.wait_op(nop.ins, seed, 0, "sem-ge", False)
            nop.ins.sync_info.on_wait = waits[i:i + _MAX_WAITS]

    self.nc.all_engine_barrier()
    assert self.sems is not None
    popped = self.nc._tile_sem_poison_stack.pop()
    assert popped is self._sem_poison
    self.nc.clear_and_free_semaphores(list(self.sems.allocated().values()))
    self.nc.all_engine_barrier()


tile.TileContext._drain_and_barrier = _split_drain_and_barrier


def _split_multi_waits(nc, max_waits=_MAX_WAITS):
    """Walrus here allows only one sem wait per instruction.  Move excess
    waits onto NoOp carriers inserted just before the instruction on the
    same engine (engines execute their program serially, so this is
    semantically identical)."""
    n_carriers = 0
    for fn in nc.m.functions:
        for blk in fn.blocks:
            out = []
            changed = False
            for inst in blk.instructions:
                si = inst.sync_info
                if si is not None and si.on_wait and len(si.on_wait) > max_waits:
                    waits = list(si.on_wait)
                    chunks = [waits[i:i + max_waits]
                              for i in range(0, len(waits), max_waits)]
                    si.on_wait = chunks[-1]
                    for ch in chunks[:-1]:
                        nop = mybir.InstNoOp(name=f"I-{nc.next_id()}",
                                             ins=[], outs=[])
                        nop.engine = inst.engine
                        nop.sync_info = _bass_rust.SyncInfo(
                            on_wait=ch, on_update=[])
                        nc.register_instruction(nop)
                        out.append(nop)
                        n_carriers += 1
                    changed = True
                out.append(inst)
            if changed:
                blk.instructions = out
    return n_carriers

N_CORES = 8
B, S, H, R, O = 4, 4096, 4096, 512, 4096
ROWS = B * S              # 16384 cached rows
RPC = ROWS // N_CORES     # 2048 rows per core
NT = RPC // 128           # 16 tiles of 128 rows
KC = R // 128             # 4 contraction chunks
OC = O // 512             # 8 output chunks
HC = H // 128             # 32 hidden chunks (B matmul)
EPS = 1e-8
QMAX = 127.0
MAGIC = 1.5 * 2**23       # round-half-even magic constant

f32 = mybir.dt.float32
bf16 = mybir.dt.bfloat16

_NC_CACHE = {}


def _build_nc():
    nc = bass.Bass()
    cached = nc.declare_dram_parameter("cached_rows", [RPC, R], f32, isOutput=False)
    at_d = nc.declare_dram_parameter("at_sw", [128, KC * O], bf16, isOutput=False)
    asc_d = nc.declare_dram_parameter("a_scale_bc", [128, O], f32, isOutput=False)
    ident_d = nc.declare_dram_parameter("ident_bf16", [128, 128], bf16, isOutput=False)
    bt_d = nc.declare_dram_parameter("bt_sw", [128, HC * R], f32, isOutput=False)
    nxt_d = nc.declare_dram_parameter("nxt_sw", [128, HC * B], f32, isOutput=False)
    out_rows_d = nc.declare_dram_parameter("out_rows", [RPC, O], f32, isOutput=True)
    out_new_d = nc.declare_dram_parameter("out_new", [B, O], f32, isOutput=True)
    nl_d = nc.declare_dram_parameter("new_latent", [B, R], f32, isOutput=True)

    with tile.TileContext(nc) as tc, ExitStack() as ctx:
        const = ctx.enter_context(tc.tile_pool(name="const", bufs=1))
        ident = const.tile([128, 128], bf16)
        nc.sync.dma_start(ident[:], ident_d[:])

        at_sb = const.tile([128, KC * O], bf16)
        for k in range(KC):
            nc.sync.dma_start(at_sb[:, k * O:(k + 1) * O],
                              at_d[:, k * O:(k + 1) * O])
        asc_bc = const.tile([128, O], f32)
        nc.sync.dma_start(asc_bc[:], asc_d[:])
        nxt_sb = const.tile([128, HC * B], f32)
        nc.sync.dma_start(nxt_sb[:], nxt_d[:])

        work = ctx.enter_context(tc.tile_pool(name="work", bufs=3))
        stat = ctx.enter_context(tc.tile_pool(name="stat", bufs=4))
        qpool = ctx.enter_context(tc.tile_pool(name="q", bufs=3))
        outp = ctx.enter_context(tc.tile_pool(name="outp", bufs=3))
        psum_t = ctx.enter_context(tc.tile_pool(name="psum_t", bufs=2, space="PSUM"))
        psum_o = ctx.enter_context(tc.tile_pool(name="psum_o", bufs=5, space="PSUM"))
        psum_nl = ctx.enter_context(tc.tile_pool(name="psum_nl", bufs=1, space="PSUM"))

        def quant_rows(L, P, tagsuf):
            """Quantize rows of L [P, R] -> (q bf16 [P,R], sdeq f32 [P,1])."""
            absmax = stat.tile([P, 1], f32, tag="absmax" + tagsuf)
            nc.vector.tensor_reduce(
                absmax[:], L[:], axis=mybir.AxisListType.X,
                op=mybir.AluOpType.max, apply_absolute_value=True,
            )
            amax = stat.tile([P, 1], f32, tag="amax" + tagsuf)
            nc.vector.tensor_scalar_max(amax[:], absmax[:], EPS)
            inv = stat.tile([P, 1], f32, tag="inv" + tagsuf)
            nc.vector.reciprocal(inv[:], amax[:])
            inv127 = stat.tile([P, 1], f32, tag="inv127" + tagsuf)
            nc.vector.tensor_scalar_mul(inv127[:], inv[:], QMAX)
            sdeq = stat.tile([P, 1], f32, tag="sdeq" + tagsuf)
            nc.vector.tensor_scalar_mul(sdeq[:], amax[:], 1.0 / QMAX)
            t = work.tile([P, R], f32, tag="t_round" + tagsuf)
            nc.scalar.activation(
                t[:], L[:], mybir.ActivationFunctionType.Copy,
                bias=MAGIC, scale=inv127[:],
            )
            q = qpool.tile([P, R], bf16, tag="q" + tagsuf)
            nc.vector.tensor_scalar(
                q[:], t[:], MAGIC, None, mybir.AluOpType.subtract
            )
            return q, sdeq

        # ---- new-latent path: nl = new_x @ B^T, fp32 on PE ----
        btp = ctx.enter_context(tc.tile_pool(name="btp", bufs=3))
        p_nl = psum_nl.tile([B, R], f32, tag="pnl")
        for h in range(HC):
            bt_t = btp.tile([128, R], f32, tag="bt")
            nc.sync.dma_start(bt_t[:], bt_d[:, h * R:(h + 1) * R])
            nc.tensor.matmul(
                p_nl[:],
                lhsT=nxt_sb[:, h * B:(h + 1) * B],
                rhs=bt_t[:],
                start=(h == 0), stop=(h == HC - 1),
            )
        nl_sb = const.tile([B, R], f32)
        nc.scalar.copy(nl_sb[:], p_nl[:])
        nc.sync.dma_start(nl_d[:], nl_sb[:])

        # ---- main tiles ----
        for t in range(NT):
            L = work.tile([128, R], f32, tag="L")
            nc.sync.dma_start(L[:], cached[t * 128:(t + 1) * 128, :])
            q, sdeq = quant_rows(L, 128, "")
            pt = psum_t.tile([128, 512], bf16, tag="pt")
            qt = qpool.tile([128, 512], bf16, tag="qt")
            for k in range(KC):
                sl = slice(k * 128, (k + 1) * 128)
                nc.tensor.transpose(pt[:, sl], q[:, sl], ident[:])
                nc.scalar.copy(qt[:, sl], pt[:, sl])
            out_sb = outp.tile([128, O], f32, tag="out")
            for o in range(OC):
                po = psum_o.tile([128, 512], f32, tag="po")
                for k in range(KC):
                    nc.tensor.matmul(
                        po[:],
                        lhsT=qt[:, k * 128:(k + 1) * 128],
                        rhs=at_sb[:, k * O + o * 512: k * O + (o + 1) * 512],
                        start=(k == 0), stop=(k == KC - 1),
                    )
                nc.scalar.activation(
                    out_sb[:, o * 512:(o + 1) * 512], po[:],
                    mybir.ActivationFunctionType.Copy, scale=sdeq[:],
                )
            nc.vector.tensor_mul(out_sb[:], out_sb[:], asc_bc[:])
            nc.sync.dma_start(out_rows_d[t * 128:(t + 1) * 128, :], out_sb[:])

        # ---- new-row epilogue ----
        qn, sdeq_n = quant_rows(nl_sb, B, "n")
        ptn = psum_t.tile([128, 16], bf16, tag="pt")
        qtn = qpool.tile([128, 16], bf16, tag="qtn")
        for k in range(KC):
            nc.tensor.transpose(
                ptn[:, k * B:(k + 1) * B], qn[:, k * 128:(k + 1) * 128],
                ident[:B, :B],
            )
            nc.scalar.copy(qtn[:, k * B:(k + 1) * B], ptn[:, k * B:(k + 1) * B])
        outnp = ctx.enter_context(tc.tile_pool(name="outnp", bufs=1))
        outn = outnp.tile([B, O], f32, tag="outn")
        for o in range(OC):
            pn = psum_o.tile([B, 512], f32, tag="po")
            for k in range(KC):
                nc.tensor.matmul(
                    pn[:],
                    lhsT=qtn[:, k * B:(k + 1) * B],
                    rhs=at_sb[:, k * O + o * 512: k * O + (o + 1) * 512],
                    start=(k == 0), stop=(k == KC - 1),
                )
            nc.scalar.activation(
                outn[:, o * 512:(o + 1) * 512], pn[:],
                mybir.ActivationFunctionType.Copy, scale=sdeq_n[:],
            )
        nc.vector.tensor_mul(outn[:], outn[:], asc_bc[:B, :])
        nc.sync.dma_start(out_new_d[:], outn[:])



    _split_multi_waits(nc)
    return nc


def _get_nc():
    if "nc" not in _NC_CACHE:
        _NC_CACHE["nc"] = _build_nc()
    return _NC_CACHE["nc"]


def _prep_inputs(new_x, cached_latent, B_weight, A_int8, A_scale):
    at_sw = np.ascontiguousarray(
        A_int8.T.astype(ml_dtypes.bfloat16)
        .reshape(KC, 128, O).transpose(1, 0, 2).reshape(128, KC * O)
    )
    bt_sw = np.ascontiguousarray(
        B_weight.T.reshape(HC, 128, R).transpose(1, 0, 2).reshape(128, HC * R)
    ).astype(np.float32)
    nxt_sw = np.ascontiguousarray(
        new_x[:, 0, :].T.reshape(HC, 128, B).transpose(1, 0, 2).reshape(128, HC * B)
    ).astype(np.float32)
    asc_bc = np.ascontiguousarray(
        np.broadcast_to(A_scale.reshape(1, O), (128, O))).astype(np.float32)
    ident = np.eye(128, dtype=ml_dtypes.bfloat16)
    flat_cached = cached_latent.reshape(ROWS, R)
    in_maps = []
    for c in range(N_CORES):
        in_maps.append({
            "cached_rows": np.ascontiguousarray(flat_cached[c * RPC:(c + 1) * RPC]),
            "at_sw": at_sw,
            "a_scale_bc": asc_bc,
            "ident_bf16": ident,
            "bt_sw": bt_sw,
            "nxt_sw": nxt_sw,
        })
    return in_maps


def run_device(new_x, cached_latent, B_weight, A_int8, A_scale, trace=False):
    nc = _get_nc()
    in_maps = _prep_inputs(new_x, cached_latent, B_weight, A_int8, A_scale)
    res = run_bass_kernel_spmd(nc, in_maps, list(range(N_CORES)), trace=trace)
    out = np.empty((B, S + 1, O), np.float32)
    for c in range(N_CORES):
        b0 = (c * RPC) // S
        s0 = (c * RPC) % S
        out[b0, s0:s0 + RPC, :] = res.results[c]["out_rows"]
    out[:, S, :] = res.results[0]["out_new"]
    all_latent = np.concatenate(
        [cached_latent.astype(np.float32),
         res.results[0]["new_latent"].reshape(B, 1, R)], axis=1
    )
    return (out, all_latent), res


def kernel(new_x, cached_latent, B_weight, A_int8, A_scale):
    (out, all_latent), _ = run_device(
        np.asarray(new_x), np.asarray(cached_latent), np.asarray(B_weight),
        np.asarray(A_int8), np.asarray(A_scale),
    )
    return out, all_latent


# revision 17
# speedup vs baseline: 1.4748x; 1.0622x over previous
"""Trainium2 Bass kernel for nn_LowRankValueCached.

Computation (per reference):
  new_latent = new_x @ B^T                      (4,1,512)
  all_latent = concat([cached_latent, new_latent], 1)   (4,4097,512)
  per-row symmetric int8 quant over rank=512
  out = (q @ A_int8^T) * scale * A_scale^T      (4,4097,4096) f32

Sharding: the 16384 cached rows (batch*seq flattened) are split into 8
contiguous chunks of 2048 rows, one per core.  A_int8^T (as exact bf16
ints) is replicated.  Every core also computes the tiny 4-row "new
latent" path redundantly (B^T replicated); the host takes core 0's copy.
all_latent's cached part is a passthrough of the input, assembled on host.

Quantization on device: absmax reduce -> reciprocal -> x*(127/amax) ->
round-half-even via the +/-1.5*2^23 magic-number trick -> bf16 (exact for
ints <=127).  Matmul in bf16 is exact (products sum < 2^24, fp32 accum).
Dequant: ScalarE copy-with-per-partition-scale from PSUM, then one
VectorE tensor_mul with the partition-broadcast A_scale row.
"""

import sys
from contextlib import ExitStack

import numpy as np

sys.path.insert(0, "/opt/trn_rl_repo")

import ml_dtypes  # noqa: E402

import bass_rust as _bass_rust  # noqa: E402
import concourse.bass as bass  # noqa: E402
import concourse.tile as tile  # noqa: E402
from concourse import mybir  # noqa: E402
from concourse.bass_utils import run_bass_kernel_spmd  # noqa: E402
from concourse.vector_clock import ScopedClock  # noqa: E402

# ---------------------------------------------------------------------------
# The walrus build in this container rejects instructions carrying more than
# one semaphore wait ("Too many sync wait commands" on the kernel-tail Drain).
# Split the tail drain's wait list across a chain of serial drains on the
# sync engine, which is semantically identical.
# ---------------------------------------------------------------------------
_MAX_WAITS = 1


def _split_drain_and_barrier(self, tick_clock, wait_clock):
    drain_inst = self.nc.sync.drain()
    wait_clock.add_sem_waits(
        drain_inst.ins, ScopedClock({None: tick_clock.global_clock})
    )
    si = drain_inst.ins.sync_info
    if si is not None and si.on_wait and len(si.on_wait) > _MAX_WAITS:
        waits = list(si.on_wait)
        si.on_wait = waits[:_MAX_WAITS]
        assert self.sems is not None
        seed = next(iter(self.sems.allocated().values()))
        for i in range(_MAX_WAITS, len(waits), _MAX_WAITS):
            nop = self.nc.sync.drain()
            _bass_rust.wait_op(nop.ins, seed, 0, "sem-ge", False)
            nop.ins.sync_info.on_wait = waits[i:i + _MAX_WAITS]

    self.nc.all_engine_barrier()
    assert self.sems is not None
    popped = self.nc._tile_sem_poison_stack.pop()
    assert popped is self._sem_poison
    self.nc.clear_and_free_semaphores(list(self.sems.allocated().values()))
    self.nc.all_engine_barrier()


tile.TileContext._drain_and_barrier = _split_drain_and_barrier


def _split_multi_waits(nc, max_waits=_MAX_WAITS):
    """Walrus here allows only one sem wait per instruction.  Move excess
    waits onto NoOp carriers inserted just before the instruction on the
    same engine (engines execute their program serially, so this is
    semantically identical)."""
    n_carriers = 0
    for fn in nc.m.functions:
        for blk in fn.blocks:
            out = []
            changed = False
            for inst in blk.instructions:
                si = inst.sync_info
                if si is not None and si.on_wait and len(si.on_wait) > max_waits:
                    waits = list(si.on_wait)
                    chunks = [waits[i:i + max_waits]
                              for i in range(0, len(waits), max_waits)]
                    si.on_wait = chunks[-1]
                    for ch in chunks[:-1]:
                        nop = mybir.InstNoOp(name=f"I-{nc.next_id()}",
                                             ins=[], outs=[])
                        nop.engine = inst.engine
                        nop.sync_info = _bass_rust.SyncInfo(
                            on_wait=ch, on_update=[])
                        nc.register_instruction(nop)
                        out.append(nop)
                        n_carriers += 1
                    changed = True
                out.append(inst)
            if changed:
                blk.instructions = out
    return n_carriers

N_CORES = 8
B, S, H, R, O = 4, 4096, 4096, 512, 4096
ROWS = B * S              # 16384 cached rows
RPC = ROWS // N_CORES     # 2048 rows per core
NT = RPC // 128           # 16 tiles of 128 rows
KC = R // 128             # 4 contraction chunks
OC = O // 512             # 8 output chunks
HC = H // 128             # 32 hidden chunks (B matmul)
EPS = 1e-8
QMAX = 127.0
MAGIC = 1.5 * 2**23       # round-half-even magic constant

f32 = mybir.dt.float32
bf16 = mybir.dt.bfloat16

_NC_CACHE = {}


def _build_nc():
    nc = bass.Bass()
    cached = nc.declare_dram_parameter("cached_rows", [RPC, R], f32, isOutput=False)
    at_d = nc.declare_dram_parameter("at_sw", [128, KC * O], bf16, isOutput=False)
    asc_d = nc.declare_dram_parameter("a_scale_bc", [128, O], f32, isOutput=False)
    ident_d = nc.declare_dram_parameter("ident_bf16", [128, 128], bf16, isOutput=False)
    bt_d = nc.declare_dram_parameter("bt_sw", [128, HC * R], f32, isOutput=False)
    nxt_d = nc.declare_dram_parameter("nxt_sw", [128, HC * B], f32, isOutput=False)
    out_rows_d = nc.declare_dram_parameter("out_rows", [RPC, O], f32, isOutput=True)
    out_new_d = nc.declare_dram_parameter("out_new", [B, O], f32, isOutput=True)
    nl_d = nc.declare_dram_parameter("new_latent", [B, R], f32, isOutput=True)

    with tile.TileContext(nc) as tc, ExitStack() as ctx:
        const = ctx.enter_context(tc.tile_pool(name="const", bufs=1))
        ident = const.tile([128, 128], bf16)
        nc.sync.dma_start(ident[:], ident_d[:])

        at_sb = const.tile([128, KC * O], bf16)
        for k in range(KC):
            nc.sync.dma_start(at_sb[:, k * O:(k + 1) * O],
                              at_d[:, k * O:(k + 1) * O])
        asc_bc = const.tile([128, O], f32)
        nc.sync.dma_start(asc_bc[:], asc_d[:])
        nxt_sb = const.tile([128, HC * B], f32)
        nc.sync.dma_start(nxt_sb[:], nxt_d[:])

        work = ctx.enter_context(tc.tile_pool(name="work", bufs=3))
        stat = ctx.enter_context(tc.tile_pool(name="stat", bufs=4))
        qpool = ctx.enter_context(tc.tile_pool(name="q", bufs=3))
        outp = ctx.enter_context(tc.tile_pool(name="outp", bufs=3))
        psum_t = ctx.enter_context(tc.tile_pool(name="psum_t", bufs=2, space="PSUM"))
        psum_o = ctx.enter_context(tc.tile_pool(name="psum_o", bufs=5, space="PSUM"))
        psum_nl = ctx.enter_context(tc.tile_pool(name="psum_nl", bufs=1, space="PSUM"))

        def quant_rows(L, P, tagsuf):
            """Quantize rows of L [P, R] -> (q bf16 [P,R], sdeq f32 [P,1])."""
            absmax = stat.tile([P, 1], f32, tag="absmax" + tagsuf)
            nc.vector.tensor_reduce(
                absmax[:], L[:], axis=mybir.AxisListType.X,
                op=mybir.AluOpType.max, apply_absolute_value=True,
            )
            amax = stat.tile([P, 1], f32, tag="amax" + tagsuf)
            nc.vector.tensor_scalar_max(amax[:], absmax[:], EPS)
            inv = stat.tile([P, 1], f32, tag="inv" + tagsuf)
            nc.vector.reciprocal(inv[:], amax[:])
            inv127 = stat.tile([P, 1], f32, tag="inv127" + tagsuf)
            nc.vector.tensor_scalar_mul(inv127[:], inv[:], QMAX)
            sdeq = stat.tile([P, 1], f32, tag="sdeq" + tagsuf)
            nc.vector.tensor_scalar_mul(sdeq[:], amax[:], 1.0 / QMAX)
            t = work.tile([P, R], f32, tag="t_round" + tagsuf)
            nc.scalar.activation(
                t[:], L[:], mybir.ActivationFunctionType.Copy,
                bias=MAGIC, scale=inv127[:],
            )
            q = qpool.tile([P, R], bf16, tag="q" + tagsuf)
            nc.vector.tensor_scalar(
                q[:], t[:], MAGIC, None, mybir.AluOpType.subtract
            )
            return q, sdeq

        # ---- main tiles (B-path interleaved: 2 hidden-chunks per tile) ----
        btp = ctx.enter_context(tc.tile_pool(name="btp", bufs=3))
        p_nl = psum_nl.tile([B, R], f32, tag="pnl")
        for t in range(NT):
            L = work.tile([128, R], f32, tag="L")
            nc.sync.dma_start(L[:], cached[t * 128:(t + 1) * 128, :])
            for h in range(2 * t, 2 * t + 2):
                bt_t = btp.tile([128, R], f32, tag="bt")
                nc.sync.dma_start(bt_t[:], bt_d[:, h * R:(h + 1) * R])
                nc.tensor.matmul(
                    p_nl[:],
                    lhsT=nxt_sb[:, h * B:(h + 1) * B],
                    rhs=bt_t[:],
                    start=(h == 0), stop=(h == HC - 1),
                )
            q, sdeq = quant_rows(L, 128, "")
            pt = psum_t.tile([128, 512], bf16, tag="pt")
            qt = qpool.tile([128, 512], bf16, tag="qt")
            for k in range(KC):
                sl = slice(k * 128, (k + 1) * 128)
                nc.tensor.transpose(pt[:, sl], q[:, sl], ident[:])
                nc.scalar.copy(qt[:, sl], pt[:, sl])
            out_sb = outp.tile([128, O], f32, tag="out")
            for o in range(OC):
                po = psum_o.tile([128, 512], f32, tag="po")
                for k in range(KC):
                    nc.tensor.matmul(
                        po[:],
                        lhsT=qt[:, k * 128:(k + 1) * 128],
                        rhs=at_sb[:, k * O + o * 512: k * O + (o + 1) * 512],
                        start=(k == 0), stop=(k == KC - 1),
                    )
                nc.scalar.activation(
                    out_sb[:, o * 512:(o + 1) * 512], po[:],
                    mybir.ActivationFunctionType.Copy, scale=sdeq[:],
                )
            nc.vector.tensor_mul(out_sb[:], out_sb[:], asc_bc[:])
            nc.sync.dma_start(out_rows_d[t * 128:(t + 1) * 128, :], out_sb[:])

        # ---- new-row epilogue ----
        nl_sb = const.tile([B, R], f32)
        nc.scalar.copy(nl_sb[:], p_nl[:])
        nc.sync.dma_start(nl_d[:], nl_sb[:])
        qn, sdeq_n = quant_rows(nl_sb, B, "n")
        ptn = psum_t.tile([128, 16], bf16, tag="pt")
        qtn = qpool.tile([128, 16], bf16, tag="qtn")
        for k in range(KC):
            nc.tensor.transpose(
                ptn[:, k * B:(k + 1) * B], qn[:, k * 128:(k + 1) * 128],
                ident[:B, :B],
            )
            nc.scalar.copy(qtn[:, k * B:(k + 1) * B], ptn[:, k * B:(k + 1) * B])
        outnp = ctx.enter_context(tc.tile_pool(name="outnp", bufs=1))
        outn = outnp.tile([B, O], f32, tag="outn")
        tmpn = outnp.tile([B, O], f32, tag="tmpn")
        for o in range(OC):
            osl = slice(o * 512, (o + 1) * 512)
            pn = psum_o.tile([B, 512], f32, tag="po")
            for k in range(KC):
                nc.tensor.matmul(
                    pn[:],
                    lhsT=qtn[:, k * B:(k + 1) * B],
                    rhs=at_sb[:, k * O + o * 512: k * O + (o + 1) * 512],
                    start=(k == 0), stop=(k == KC - 1),
                )
            nc.scalar.activation(
                tmpn[:, osl], pn[:],
                mybir.ActivationFunctionType.Copy, scale=sdeq_n[:],
            )
            nc.vector.tensor_mul(outn[:, osl], tmpn[:, osl], asc_bc[:B, osl])
        nc.sync.dma_start(out_new_d[:], outn[:])



    _split_multi_waits(nc)
    return nc


def _get_nc():
    if "nc" not in _NC_CACHE:
        _NC_CACHE["nc"] = _build_nc()
    return _NC_CACHE["nc"]


def _prep_inputs(new_x, cached_latent, B_weight, A_int8, A_scale):
    at_sw = np.ascontiguousarray(
        A_int8.T.astype(ml_dtypes.bfloat16)
        .reshape(KC, 128, O).transpose(1, 0, 2).reshape(128, KC * O)
    )
    bt_sw = np.ascontiguousarray(
        B_weight.T.reshape(HC, 128, R).transpose(1, 0, 2).reshape(128, HC * R)
    ).astype(np.float32)
    nxt_sw = np.ascontiguousarray(
        new_x[:, 0, :].T.reshape(HC, 128, B).transpose(1, 0, 2).reshape(128, HC * B)
    ).astype(np.float32)
    asc_bc = np.ascontiguousarray(
        np.broadcast_to(A_scale.reshape(1, O), (128, O))).astype(np.float32)
    ident = np.eye(128, dtype=ml_dtypes.bfloat16)
    flat_cached = cached_latent.reshape(ROWS, R)
    in_maps = []
    for c in range(N_CORES):
        in_maps.append({
            "cached_rows": np.ascontiguousarray(flat_cached[c * RPC:(c + 1) * RPC]),
            "at_sw": at_sw,
            "a_scale_bc": asc_bc,
            "ident_bf16": ident,
            "bt_sw": bt_sw,
            "nxt_sw": nxt_sw,
        })
    return in_maps


def run_device(new_x, cached_latent, B_weight, A_int8, A_scale, trace=False):
    nc = _get_nc()
    in_maps = _prep_inputs(new_x, cached_latent, B_weight, A_int8, A_scale)
    res = run_bass_kernel_spmd(nc, in_maps, list(range(N_CORES)), trace=trace)
    out = np.empty((B, S + 1, O), np.float32)
    for c in range(N_CORES):
        b0 = (c * RPC) // S
        s0 = (c * RPC) % S
        out[b0, s0:s0 + RPC, :] = res.results[c]["out_rows"]
    out[:, S, :] = res.results[0]["out_new"]
    all_latent = np.concatenate(
        [cached_latent.astype(np.float32),
         res.results[0]["new_latent"].reshape(B, 1, R)], axis=1
    )
    return (out, all_latent), res


def kernel(new_x, cached_latent, B_weight, A_int8, A_scale):
    (out, all_latent), _ = run_device(
        np.asarray(new_x), np.asarray(cached_latent), np.asarray(B_weight),
        np.asarray(A_int8), np.asarray(A_scale),
    )
    return out, all_latent
